# revision 33
# baseline (speedup 1.0000x reference)
"""BrainGNN (3-layer GCN + mean-pool + MLP head) on 8 Trainium2 cores.

Sharding: destination nodes (and their incident edges) are partitioned
across the 8 cores; each layer all-gathers the projected node-feature
table, gathers source rows per edge via dma_gather (4 int16-safe source
banks), reduces edge messages with one-hot bf16 matmuls on the
TensorEngine (GCN edge weights folded into the one-hot values), writes
the per-chunk partial sums DENSELY to an HBM staging area, and reads
them back in destination order with a second dma_gather (each dst lives
in exactly one chunk per bank, so no scatter-add is ever needed; dsts
with no edges in a bank read a dedicated zero row). BatchNorm (eval) is
folded into the weights. The per-graph mean-pool is a matmul against a
1/cnt-weighted graph one-hot, finished with an AllReduce, and the MLP
head runs replicated on every core.
"""
import contextlib
import ctypes
import sys
import types

import numpy as np

for _p in ("/opt/trn_rl_repo", "/root/.axon_site/_ro/trn_rl_repo"):
    if _p not in sys.path:
        sys.path.append(_p)

# ---------------------------------------------------------------- constants
N = 100000
E = 3200000
F = 64
G = 16
C = 8            # cores
NPC = N // C     # 12500 nodes per core
NPCP = 12544     # padded to 98*128
NSB = NPCP // 128  # 98 row-tiles per shard
NBANK = 4
HALF = 6272       # half-shard rows (49 tiles); AllGather is split per half
BANKR = 4 * HALF  # 25088 table rows per bank (4 ranks x one half-shard)
BN_EPS = 1e-5
CHUNK = 128      # slots per chunk
MAXD = 16        # distinct dsts per chunk
BCH = 64         # chunks per batch
GNUM = BCH * CHUNK   # 8192 gather idxs per batch
SLOTM = 32       # staged rows per chunk (16 real + 16 holes, 32-aligned PSUM)
SNUM = BCH * SLOTM   # 2048 staged rows per batch
RBH = NPCP // 2      # 6272 readback idxs per half-shard

_SO_PATH = "/opt/axon/libaxon_pjrt.so"


def _install_axon_prof_hook():
    """bass_utils needs antenv.axon_hooks for trace=True under axon."""
    if "antenv.axon_hooks" in sys.modules:
        return
    try:
        lib = ctypes.CDLL(_SO_PATH)
    except OSError:
        lib = None
    hook = None
    if lib is not None and hasattr(lib, "axon_start_nrt_profile"):
        lib.axon_start_nrt_profile.argtypes = [
            ctypes.POINTER(ctypes.c_int64),
            ctypes.c_size_t,
        ]
        lib.axon_start_nrt_profile.restype = ctypes.c_int64
        lib.axon_stop_nrt_profile.argtypes = [ctypes.c_char_p]
        lib.axon_stop_nrt_profile.restype = ctypes.c_int64

        @contextlib.contextmanager
        def _hook(output_dir, device_ids):
            import jax

            jax.devices()
            if device_ids:
                ids = (ctypes.c_int64 * len(device_ids))(*device_ids)
                rc = lib.axon_start_nrt_profile(ids, len(device_ids))
            else:
                rc = lib.axon_start_nrt_profile(None, 0)
            if rc != 0:
                raise RuntimeError(f"axon_start_nrt_profile rc={rc}")
            try:
                yield
            finally:
                n = lib.axon_stop_nrt_profile(str(output_dir).encode())
                print(f"profile: {n} file(s) in {output_dir}", file=sys.stderr)

        hook = _hook

    mod = types.ModuleType("antenv.axon_hooks")
    mod.get_axon_ntff_profile_hook = lambda: hook
    mod.set_axon_ntff_profile_hook = lambda h: None
    sys.modules["antenv.axon_hooks"] = mod

    from concourse import bass_utils

    bass_utils.upload_artifacts = lambda tmpdir: f"file://{tmpdir}"


# ---------------------------------------------------------------- host plan
def _pack_idx16(vals, ncols):
    """Index j -> (partition j%16 replicated x8, col j//16)."""
    out = np.zeros((128, ncols), np.int16)
    n = len(vals)
    cols = max(1, (n + 15) // 16)
    tmp = np.zeros(16 * cols, np.int16)
    tmp[:n] = vals
    blk = tmp.reshape(cols, 16).T  # [16, cols]
    out[:, :cols] = np.tile(blk, (8, 1))
    return out


def build_plan(edge_index, edge_weight, batch):
    ei = np.asarray(edge_index)
    ew = np.asarray(edge_weight, np.float64)
    bt = np.asarray(batch).astype(np.int64)

    row = np.concatenate([ei[0], np.arange(N, dtype=ei.dtype)]).astype(np.int64)
    col = np.concatenate([ei[1], np.arange(N, dtype=ei.dtype)]).astype(np.int64)
    w = np.concatenate([ew, np.ones(N, np.float64)])

    deg = np.bincount(col, weights=w, minlength=N)
    dis = np.where(deg > 0, 1.0 / np.sqrt(np.maximum(deg, 1e-30)), 0.0)
    val = (dis[row] * w * dis[col]).astype(np.float32)

    core = col // NPC
    # table = [AG0: ranks x half0 | AG1: ranks x half1], each AG output
    # split into two banks of 4 ranks; bank b = 2*(half) + (rank >= 4)
    s = row // NPC
    i = row % NPC
    bank = 2 * (i // HALF) + (s >= 4)
    lsrc = ((s % 4) * HALF + i % HALF).astype(np.int64)
    ldst = (col % NPC).astype(np.int64)

    # per (core, bank): edges sorted by local dst
    per_cb = {}
    for c in range(C):
        mc = core == c
        for b in range(NBANK):
            m = mc & (bank == b)
            ld, ls, v = ldst[m], lsrc[m], val[m]
            o = np.argsort(ld, kind="stable")
            per_cb[(c, b)] = (ld[o], ls[o], v[o])

    # chunking: whole dsts, <=128 slots, <=16 distinct dsts
    chunks_cb = {}
    for (c, b), (ld, ls, v) in per_cb.items():
        dst_u, dst_start, dst_cnt = np.unique(ld, return_index=True, return_counts=True)
        assert dst_cnt.max(initial=0) <= CHUNK, "dst bank-degree exceeds chunk size"
        csum = np.concatenate([[0], np.cumsum(dst_cnt)])
        chunks = []  # (dst_lo_i, dst_hi_i) index range into dst_u
        i = 0
        nd = len(dst_u)
        while i < nd:
            # max j with csum[j]-csum[i] <= 128 and j-i <= 16
            j = np.searchsorted(csum, csum[i] + CHUNK, side="right") - 1
            j = min(j, i + MAXD, nd)
            assert j > i
            chunks.append((i, j))
            i = j
        chunks_cb[(c, b)] = (dst_u, csum, chunks, ld, ls, v)

    nbatch = 0
    for (c, b), (_, _, chunks, _, _, _) in chunks_cb.items():
        nbatch = max(nbatch, (len(chunks) + BCH - 1) // BCH)
    nchunks = nbatch * BCH
    zrow = nbatch * SNUM  # staged zero row per bank
    assert zrow <= 32767, f"staged rows {zrow} exceed int16 gather range"

    # build per-core arrays
    plans = []
    for c in range(C):
        gidx = np.zeros((128, NBANK * nbatch * GNUM // 16), np.int16)
        onehot = np.zeros((128, NBANK * nchunks, MAXD), np.float32)
        rbidx = np.zeros((128, NBANK * 2 * (RBH // 16)), np.int16)
        for b in range(NBANK):
            dst_u, csum, chunks, ld, ls, v = chunks_cb[(c, b)]
            # pad = 0: a safe in-bank read whose one-hot column is zero
            # (negative "skip" indices hang the device — twice reproduced)
            gvals = np.zeros(nbatch * GNUM, np.int64)
            rbrow = np.full(NPCP, zrow, np.int64)  # default: zero row
            for k, (i, j) in enumerate(chunks):
                e0, e1 = csum[i], csum[j]
                nsl = e1 - e0
                # order slots by source row for HBM read locality
                perm = np.argsort(ls[e0:e1], kind="stable")
                gvals[k * CHUNK:k * CHUNK + nsl] = ls[e0:e1][perm]
                q, kk = divmod(k, BCH)
                h, ksl = divmod(kk, 4)
                base = q * SNUM + h * 128 + ksl * 32
                rbrow[dst_u[i:j]] = base + np.arange(j - i)
                # one-hot columns: position within chunk's distinct dsts
                colid = np.searchsorted(dst_u[i:j], ld[e0:e1])[perm]
                oh = onehot[:, b * nchunks + k, :]
                oh[np.arange(nsl), colid] = v[e0:e1][perm]
            q0 = b * nbatch
            gidx[:, q0 * (GNUM // 16):(q0 + nbatch) * (GNUM // 16)] = _pack_idx16(
                gvals, nbatch * GNUM // 16)
            for half in range(2):
                col0 = (b * 2 + half) * (RBH // 16)
                rbidx[:, col0:col0 + RBH // 16] = _pack_idx16(
                    rbrow[half * RBH:(half + 1) * RBH], RBH // 16)

        # graph pooling one-hot with 1/cnt
        cnt = np.bincount(bt, minlength=G).astype(np.float64)
        inv = (1.0 / np.maximum(cnt, 1.0)).astype(np.float32)
        gpool = np.zeros((128, NSB, G), np.float32)
        nodes = np.arange(NPC) + c * NPC
        gb = bt[nodes]
        p = np.arange(NPC) % 128
        sb = np.arange(NPC) // 128
        gpool[p, sb, gb] = inv[gb]
        plans.append(dict(gidx=gidx, onehot=onehot, rbidx=rbidx, gpool=gpool))

    return plans, nbatch, nchunks


def _fold_weights(inputs):
    s = 1.0 / np.float32(np.sqrt(1.0 + BN_EPS))
    Ws, bs = inputs["Ws"], inputs["bs"]
    bn_g, bn_b = inputs["bn_g"], inputs["bn_b"]
    conv = []
    for l in range(3):
        sl = (np.asarray(bn_g[l]) * s).astype(np.float32)
        Wp = (np.asarray(Ws[l]) * sl[None, :]).astype(np.float32)
        bp = (np.asarray(bs[l]) * sl + np.asarray(bn_b[l])).astype(np.float32)
        conv.append((Wp, bp))
    s1 = (np.asarray(inputs["fc1_g"]) * s).astype(np.float32)
    W1 = (np.asarray(inputs["fc1_W"]) * s1[None, :]).astype(np.float32)
    b1 = (np.asarray(inputs["fc1_b"]) * s1 + np.asarray(inputs["fc1_bt"])).astype(np.float32)
    s2 = (np.asarray(inputs["fc2_g"]) * s).astype(np.float32)
    W2 = (np.asarray(inputs["fc2_W"]) * s2[None, :]).astype(np.float32)
    b2 = (np.asarray(inputs["fc2_b"]) * s2 + np.asarray(inputs["fc2_bt"])).astype(np.float32)
    Wo = np.asarray(inputs["fco_W"], np.float32)
    bo = np.asarray(inputs["fco_b"], np.float32)
    return conv, (W1, b1), (W2, b2), (Wo, bo)


# ---------------------------------------------------------------- device
def build_bass(nbatch, nchunks):
    import concourse.bacc as bacc
    import concourse.bass as bass
    import concourse.mybir as mybir
    import concourse.tile as tile

    dt = mybir.dt
    nc = bacc.Bacc("TRN2", target_bir_lowering=False, debug=False, num_devices=C,
                   num_swdge_queues=4)

    zrow = nbatch * SNUM

    xT_in = nc.dram_tensor("xT", [F, NPCP], dt.bfloat16, kind="ExternalInput")
    gidx_in = nc.dram_tensor("gidx", [128, NBANK * nbatch * GNUM // 16], dt.int16,
                             kind="ExternalInput")
    oh_in = nc.dram_tensor("onehot", [128, NBANK * nchunks, MAXD], dt.bfloat16,
                           kind="ExternalInput")
    rbidx_in = nc.dram_tensor("rbidx", [128, NBANK * 2 * (RBH // 16)], dt.int16,
                              kind="ExternalInput")
    gpool_in = nc.dram_tensor("gpool", [128, NSB, G], dt.float32, kind="ExternalInput")
    Wc_in = nc.dram_tensor("Wconv", [3, F, F], dt.bfloat16, kind="ExternalInput")
    bc_in = nc.dram_tensor("bconv", [3, 128, F], dt.float32, kind="ExternalInput")
    W1_in = nc.dram_tensor("W1", [F, F], dt.float32, kind="ExternalInput")
    b1_in = nc.dram_tensor("b1", [F, 1], dt.float32, kind="ExternalInput")
    W2_in = nc.dram_tensor("W2", [F, 32], dt.float32, kind="ExternalInput")
    b2_in = nc.dram_tensor("b2", [32, 1], dt.float32, kind="ExternalInput")
    Wo_in = nc.dram_tensor("Wo", [32, 2], dt.float32, kind="ExternalInput")
    bo_in = nc.dram_tensor("bo", [2, 1], dt.float32, kind="ExternalInput")
    ident_in = nc.dram_tensor("ident", [128, 128], dt.float32, kind="ExternalInput")
    out_ext = nc.dram_tensor("out", [2, G], dt.float32, kind="ExternalOutput")

    # table rows are 64 feats duplicated twice (256B bf16) so dma_gather's
    # 256B-min element lands directly in matmul-ready bf16
    shard_d = nc.dram_tensor("shard_d", [NPCP, 2 * F], dt.bfloat16)
    table = nc.dram_tensor("table", [C * NPCP, 2 * F], dt.bfloat16,
                           addr_space="Shared")
    staged = nc.dram_tensor("staged", [NBANK, zrow + 1, F], dt.float32)
    pool_in_d = nc.dram_tensor("pool_in", [F, G], dt.float32)
    pool_out_d = nc.dram_tensor("pool_out", [F, G], dt.float32, addr_space="Shared")

    RG = [list(range(C))]

    with tile.TileContext(nc) as tc:
        with (
            tc.tile_pool(name="persist", bufs=1) as persist,
            tc.tile_pool(name="ht", bufs=2) as htp,
            tc.tile_pool(name="io", bufs=6) as iop,
            tc.tile_pool(name="msgp", bufs=4) as msgp,
            tc.tile_pool(name="stagep", bufs=2) as stagep,
            tc.tile_pool(name="rbp", bufs=1) as rbp,
            tc.tile_pool(name="small", bufs=4) as smallp,
            tc.tile_pool(name="ppsum", bufs=2, space="PSUM") as ppsum,
            tc.tile_pool(name="spsum", bufs=3, space="PSUM") as spsum,
            tc.tile_pool(name="tpsum", bufs=2, space="PSUM") as tpsum,
            tc.tile_pool(name="accpsum", bufs=1, space="PSUM") as accpsum,
        ):
            ident = persist.tile([128, 128], dt.float32)
            nc.sync.dma_start(ident[:], ident_in[:])
            Wc = persist.tile([F, 3 * F], dt.bfloat16)
            nc.sync.dma_start(Wc[:].rearrange("p (l f) -> p l f", l=3),
                              Wc_in[:].rearrange("l p f -> p l f"))
            bc = persist.tile([128, 3 * F], dt.float32)
            nc.sync.dma_start(bc[:].rearrange("p (l f) -> p l f", l=3),
                              bc_in[:].rearrange("l p f -> p l f"))
            gpool = persist.tile([128, NSB * G], dt.float32)
            nc.sync.dma_start(gpool[:].rearrange("p (s g) -> p s g", g=G), gpool_in[:])
            zz = persist.tile([1, F], dt.float32)
            nc.vector.memset(zz[:], 0.0)
            for b in range(NBANK):
                nc.sync.dma_start(staged[b, zrow:zrow + 1, :], zz[:])

            # pre-zero the msg ring slots so slots skipped by padded
            # gathers never hold non-finite garbage (0 * one-hot stays 0)
            for _ in range(4):
                mz = msgp.tile([128, BCH * 2 * F], dt.bfloat16, tag="msg")
                nc.gpsimd.memset(mz[:], 0.0)

            hT = htp.tile([F, NPCP], dt.bfloat16, name="hT0", tag="hT")
            nc.sync.dma_start(hT[:], xT_in[:])

            def proj_half(l, half, hsrc):
                # project one half-shard and allgather it: the AG overlaps
                # the other half's readback/merge and the early banks' batches
                for t in range(half * (NSB // 2), (half + 1) * (NSB // 2)):
                    pp = ppsum.tile([128, F], dt.float32, space="PSUM", tag="pp")
                    nc.tensor.matmul(out=pp[:], lhsT=hsrc[:, t * 128:(t + 1) * 128],
                                     rhs=Wc[:, l * F:(l + 1) * F],
                                     start=True, stop=True)
                    ps = smallp.tile([128, 2 * F], dt.bfloat16, tag="ps")
                    nc.vector.tensor_copy(ps[:, 0:F], pp[:])
                    nc.scalar.activation(ps[:, F:2 * F], pp[:],
                                         mybir.ActivationFunctionType.Copy)
                    nc.sync.dma_start(shard_d[t * 128:(t + 1) * 128, :], ps[:])
                nc.gpsimd.collective_compute(
                    "AllGather", mybir.AluOpType.bypass, replica_groups=RG,
                    ins=[shard_d[half * HALF:(half + 1) * HALF]],
                    outs=[table[half * 8 * HALF:(half + 1) * 8 * HALF]])

            proj_half(0, 0, hT)
            proj_half(0, 1, hT)

            qn = 0
            for l in range(3):
                # ---- gather / one-hot matmul / dense staging
                # (banks 0/1 depend on AG half 0 only, so issue them first;
                # bank interleave spreads the sparse trailing batches)
                for gi in range(2 * nbatch):
                    g = gi % nbatch
                    for b in (0, 1) if gi < nbatch else (2, 3):
                        q = b * nbatch + g
                        git = iop.tile([128, GNUM // 16], dt.int16, tag="git")
                        nc.sync.dma_start(
                            git[:], gidx_in[:, q * (GNUM // 16):(q + 1) * (GNUM // 16)])
                        msg = msgp.tile([128, BCH * 2 * F], dt.bfloat16, tag="msg")
                        nc.gpsimd.dma_gather(
                            out_ap=msg[:].rearrange("p (c f) -> p c f", f=2 * F),
                            in_ap=table[b * BANKR:(b + 1) * BANKR, :],
                            idxs_ap=git[:],
                            num_idxs=GNUM, num_idxs_reg=GNUM, elem_size=2 * F,
                            single_packet=False,
                            queue_num=qn % 4)
                        qn += 1
                        oht = iop.tile([128, BCH * MAXD], dt.bfloat16, tag="oht")
                        nc.sync.dma_start(
                            oht[:].rearrange("p (c m) -> p c m", m=MAXD),
                            oh_in[:, q * BCH:(q + 1) * BCH, :])
                        stg_s = stagep.tile([128, (SNUM // 128) * F], dt.float32,
                                            tag="stg")
                        for h in range(SNUM // 128):
                            sp = spsum.tile([128, F], dt.float32, space="PSUM",
                                            tag="sp")
                            for k in range(4):
                                kk = h * 4 + k
                                nc.tensor.matmul(
                                    out=sp[k * 32:k * 32 + MAXD, :],
                                    lhsT=oht[:, kk * MAXD:(kk + 1) * MAXD],
                                    rhs=msg[:, kk * 2 * F:kk * 2 * F + F],
                                    start=True, stop=True,
                                    tile_position=(0, k * 32))
                            if h % 2 == 0:
                                nc.scalar.activation(
                                    stg_s[:, h * F:(h + 1) * F], sp[:],
                                    mybir.ActivationFunctionType.Copy)
                            else:
                                nc.vector.tensor_copy(
                                    stg_s[:, h * F:(h + 1) * F], sp[:])
                        # on the scalar queue: a staging write waits on this
                        # batch's PSUM copies, and on the sync queue it would
                        # head-of-line block the next batches' index loads
                        nc.scalar.dma_start(
                            staged[b, g * SNUM:(g + 1) * SNUM, :].rearrange(
                                "(h p) f -> p h f", p=128),
                            stg_s[:].rearrange("p (h f) -> p h f", f=F))

                # ---- readback: gather staged rows into dst order, merge
                # banks, bias + relu (+ transpose | pool)
                if l < 2:
                    hT_next = htp.tile([F, NPCP], dt.bfloat16, name=f"hT{l + 1}",
                                       tag="hT")
                else:
                    pacc = accpsum.tile([F, G], dt.float32, space="PSUM")
                for half in range(2):
                    rbt = []
                    for b in range(NBANK):
                        rb = rbp.tile([128, (RBH // 128) * F], dt.float32,
                                      tag=f"rb{b}")
                        col0 = (b * 2 + half) * (RBH // 16)
                        rit = iop.tile([128, RBH // 16], dt.int16, tag="rit")
                        nc.sync.dma_start(
                            rit[:], rbidx_in[:, col0:col0 + RBH // 16])
                        nc.gpsimd.dma_gather(
                            out_ap=rb[:].rearrange("p (c f) -> p c f", f=F),
                            in_ap=staged[b],
                            idxs_ap=rit[:],
                            num_idxs=RBH, num_idxs_reg=RBH, elem_size=F,
                            single_packet=False,
                            queue_num=qn % 4)
                        qn += 1
                        rbt.append(rb)
                    for tl in range(RBH // 128):
                        t = half * (RBH // 128) + tl
                        sl = slice(tl * F, (tl + 1) * F)
                        m0 = smallp.tile([128, F], dt.float32, tag="m0")
                        nc.vector.tensor_add(m0[:], rbt[0][:, sl], rbt[1][:, sl])
                        m1 = smallp.tile([128, F], dt.float32, tag="m1")
                        nc.vector.tensor_add(m1[:], rbt[2][:, sl], rbt[3][:, sl])
                        nc.vector.tensor_add(m0[:], m0[:], m1[:])
                        nc.vector.tensor_add(
                            m0[:], m0[:], bc[:, l * F:(l + 1) * F])
                        nc.vector.tensor_scalar_max(m0[:], m0[:], 0.0)
                        if l < 2:
                            pt = tpsum.tile([F, 128], dt.float32, space="PSUM",
                                            tag="pt")
                            nc.tensor.transpose(pt[:], m0[:], ident[:])
                            nc.scalar.activation(
                                hT_next[:, t * 128:(t + 1) * 128], pt[:],
                                mybir.ActivationFunctionType.Copy)
                        else:
                            nc.tensor.matmul(
                                out=pacc[:], lhsT=m0[:],
                                rhs=gpool[:, t * G:(t + 1) * G],
                                start=(t == 0), stop=(t == NSB - 1))
                    # next layer's projection+AG of this half starts now,
                    # overlapping the other half's readback
                    if l < 2:
                        proj_half(l + 1, half, hT_next)
                if l < 2:
                    hT = hT_next

            # ---- pooled AllReduce + MLP head
            pool_s = smallp.tile([F, G], dt.float32, tag="pool_s")
            nc.vector.tensor_copy(pool_s[:], pacc[:])
            nc.sync.dma_start(pool_in_d[:], pool_s[:])
            nc.gpsimd.collective_compute(
                "AllReduce", mybir.AluOpType.add, replica_groups=RG,
                ins=[pool_in_d[:]], outs=[pool_out_d[:]])
            pooled = smallp.tile([F, G], dt.float32, tag="pooled")
            nc.sync.dma_start(pooled[:], pool_out_d[:])

            W1t = smallp.tile([F, F], dt.float32, tag="W1t")
            nc.sync.dma_start(W1t[:], W1_in[:])
            b1t = smallp.tile([F, 1], dt.float32, tag="b1t")
            nc.sync.dma_start(b1t[:], b1_in[:])
            W2t = smallp.tile([F, 32], dt.float32, tag="W2t")
            nc.sync.dma_start(W2t[:], W2_in[:])
            b2t = smallp.tile([32, 1], dt.float32, tag="b2t")
            nc.sync.dma_start(b2t[:], b2_in[:])
            Wot = smallp.tile([32, 2], dt.float32, tag="Wot")
            nc.sync.dma_start(Wot[:], Wo_in[:])
            bot = smallp.tile([2, 1], dt.float32, tag="bot")
            nc.sync.dma_start(bot[:], bo_in[:])

            h1p = ppsum.tile([F, G], dt.float32, space="PSUM", tag="pp")
            nc.tensor.matmul(out=h1p[:], lhsT=W1t[:], rhs=pooled[:],
                             start=True, stop=True)
            h1 = smallp.tile([F, G], dt.float32, tag="h1")
            nc.scalar.activation(h1[:], h1p[:], mybir.ActivationFunctionType.Relu,
                                 bias=b1t[:])
            h2p = ppsum.tile([32, G], dt.float32, space="PSUM", tag="pp")
            nc.tensor.matmul(out=h2p[:], lhsT=W2t[:], rhs=h1[:],
                             start=True, stop=True)
            h2 = smallp.tile([32, G], dt.float32, tag="h2")
            nc.scalar.activation(h2[:], h2p[:], mybir.ActivationFunctionType.Relu,
                                 bias=b2t[:])
            hop = ppsum.tile([2, G], dt.float32, space="PSUM", tag="pp")
            nc.tensor.matmul(out=hop[:], lhsT=Wot[:], rhs=h2[:],
                             start=True, stop=True)
            outt = smallp.tile([2, G], dt.float32, tag="outt")
            nc.vector.tensor_add(outt[:], hop[:], bot[:].to_broadcast([2, G]))
            nc.sync.dma_start(out_ext[:], outt[:])

    nc.compile()
    return nc


_CACHE = {}


def kernel(**inputs) -> np.ndarray:
    _install_axon_prof_hook()
    import ml_dtypes
    from concourse.bass_utils import run_bass_kernel_spmd

    x = np.asarray(inputs["x"], np.float32)
    plans, nbatch, nchunks = build_plan(
        inputs["edge_index"], inputs["edge_weight"], inputs["batch"])
    conv, (W1, b1), (W2, b2), (Wo, bo) = _fold_weights(inputs)

    key = (nbatch, nchunks)
    if key not in _CACHE:
        _CACHE[key] = build_bass(nbatch, nchunks)
    nc = _CACHE[key]

    Wconv = np.stack([c[0] for c in conv]).astype(ml_dtypes.bfloat16)  # [3, F, F]
    bconv = np.stack([np.broadcast_to(c[1], (128, F)) for c in conv]).copy()
    ident = np.eye(128, dtype=np.float32)

    in_maps = []
    for c in range(C):
        xT = np.zeros((F, NPCP), np.float32)
        xT[:, :NPC] = x[c * NPC:(c + 1) * NPC].T
        in_maps.append({
            "xT": xT.astype(ml_dtypes.bfloat16),
            "gidx": plans[c]["gidx"],
            "onehot": plans[c]["onehot"].astype(ml_dtypes.bfloat16),
            "rbidx": plans[c]["rbidx"],
            "gpool": plans[c]["gpool"],
            "Wconv": Wconv, "bconv": bconv,
            "W1": W1, "b1": b1[:, None],
            "W2": W2, "b2": b2[:, None],
            "Wo": Wo, "bo": bo[:, None],
            "ident": ident,
        })

    trace = bool(int(__import__("os").environ.get("BGNN_TRACE", "0")))
    res = run_bass_kernel_spmd(nc, in_maps, list(range(C)), trace=trace)
    kernel.last_exec_time_ns = res.exec_time_ns
    return np.ascontiguousarray(res.results[0]["out"].T)


kernel.last_exec_time_ns = None


# revision 36
# speedup vs baseline: 1.0588x; 1.0588x over previous
"""BrainGNN (3-layer GCN + mean-pool + MLP head) on 8 Trainium2 cores.

Sharding: destination nodes (and their incident edges) are partitioned
across the 8 cores; each layer all-gathers the projected node-feature
table, gathers source rows per edge via dma_gather (4 int16-safe source
banks), reduces edge messages with one-hot bf16 matmuls on the
TensorEngine (GCN edge weights folded into the one-hot values), writes
the per-chunk partial sums DENSELY to an HBM staging area, and reads
them back in destination order with a second dma_gather (each dst lives
in exactly one chunk per bank, so no scatter-add is ever needed; dsts
with no edges in a bank read a dedicated zero row). BatchNorm (eval) is
folded into the weights. The per-graph mean-pool is a matmul against a
1/cnt-weighted graph one-hot, finished with an AllReduce, and the MLP
head runs replicated on every core.
"""
import contextlib
import ctypes
import sys
import types

import numpy as np

for _p in ("/opt/trn_rl_repo", "/root/.axon_site/_ro/trn_rl_repo"):
    if _p not in sys.path:
        sys.path.append(_p)

# ---------------------------------------------------------------- constants
N = 100000
E = 3200000
F = 64
G = 16
C = 8            # cores
NPC = N // C     # 12500 nodes per core
NPCP = 12544     # padded to 98*128
NSB = NPCP // 128  # 98 row-tiles per shard
NBANK = 4
HALF = 6272       # half-shard rows (49 tiles); AllGather is split per half
BANKR = 4 * HALF  # 25088 table rows per bank (4 ranks x one half-shard)
BN_EPS = 1e-5
CHUNK = 128      # slots per chunk
MAXD = 16        # distinct dsts per chunk
BCH = 48         # chunks per batch
GNUM = BCH * CHUNK   # 8192 gather idxs per batch
SLOTM = 32       # staged rows per chunk (16 real + 16 holes, 32-aligned PSUM)
SNUM = BCH * SLOTM   # 2048 staged rows per batch
RBH = NPCP // 2      # 6272 readback idxs per half-shard

_SO_PATH = "/opt/axon/libaxon_pjrt.so"


def _install_axon_prof_hook():
    """bass_utils needs antenv.axon_hooks for trace=True under axon."""
    if "antenv.axon_hooks" in sys.modules:
        return
    try:
        lib = ctypes.CDLL(_SO_PATH)
    except OSError:
        lib = None
    hook = None
    if lib is not None and hasattr(lib, "axon_start_nrt_profile"):
        lib.axon_start_nrt_profile.argtypes = [
            ctypes.POINTER(ctypes.c_int64),
            ctypes.c_size_t,
        ]
        lib.axon_start_nrt_profile.restype = ctypes.c_int64
        lib.axon_stop_nrt_profile.argtypes = [ctypes.c_char_p]
        lib.axon_stop_nrt_profile.restype = ctypes.c_int64

        @contextlib.contextmanager
        def _hook(output_dir, device_ids):
            import jax

            jax.devices()
            if device_ids:
                ids = (ctypes.c_int64 * len(device_ids))(*device_ids)
                rc = lib.axon_start_nrt_profile(ids, len(device_ids))
            else:
                rc = lib.axon_start_nrt_profile(None, 0)
            if rc != 0:
                raise RuntimeError(f"axon_start_nrt_profile rc={rc}")
            try:
                yield
            finally:
                n = lib.axon_stop_nrt_profile(str(output_dir).encode())
                print(f"profile: {n} file(s) in {output_dir}", file=sys.stderr)

        hook = _hook

    mod = types.ModuleType("antenv.axon_hooks")
    mod.get_axon_ntff_profile_hook = lambda: hook
    mod.set_axon_ntff_profile_hook = lambda h: None
    sys.modules["antenv.axon_hooks"] = mod

    from concourse import bass_utils

    bass_utils.upload_artifacts = lambda tmpdir: f"file://{tmpdir}"


# ---------------------------------------------------------------- host plan
def _pack_idx16(vals, ncols):
    """Index j -> (partition j%16 replicated x8, col j//16)."""
    out = np.zeros((128, ncols), np.int16)
    n = len(vals)
    cols = max(1, (n + 15) // 16)
    tmp = np.zeros(16 * cols, np.int16)
    tmp[:n] = vals
    blk = tmp.reshape(cols, 16).T  # [16, cols]
    out[:, :cols] = np.tile(blk, (8, 1))
    return out


def build_plan(edge_index, edge_weight, batch):
    ei = np.asarray(edge_index)
    ew = np.asarray(edge_weight, np.float64)
    bt = np.asarray(batch).astype(np.int64)

    row = np.concatenate([ei[0], np.arange(N, dtype=ei.dtype)]).astype(np.int64)
    col = np.concatenate([ei[1], np.arange(N, dtype=ei.dtype)]).astype(np.int64)
    w = np.concatenate([ew, np.ones(N, np.float64)])

    deg = np.bincount(col, weights=w, minlength=N)
    dis = np.where(deg > 0, 1.0 / np.sqrt(np.maximum(deg, 1e-30)), 0.0)
    val = (dis[row] * w * dis[col]).astype(np.float32)

    core = col // NPC
    # table = [AG0: ranks x half0 | AG1: ranks x half1], each AG output
    # split into two banks of 4 ranks; bank b = 2*(half) + (rank >= 4)
    s = row // NPC
    i = row % NPC
    bank = 2 * (i // HALF) + (s >= 4)
    lsrc = ((s % 4) * HALF + i % HALF).astype(np.int64)
    ldst = (col % NPC).astype(np.int64)

    # per (core, bank): edges sorted by local dst
    per_cb = {}
    for c in range(C):
        mc = core == c
        for b in range(NBANK):
            m = mc & (bank == b)
            ld, ls, v = ldst[m], lsrc[m], val[m]
            o = np.argsort(ld, kind="stable")
            per_cb[(c, b)] = (ld[o], ls[o], v[o])

    # chunking: whole dsts, <=128 slots, <=16 distinct dsts
    chunks_cb = {}
    for (c, b), (ld, ls, v) in per_cb.items():
        dst_u, dst_start, dst_cnt = np.unique(ld, return_index=True, return_counts=True)
        assert dst_cnt.max(initial=0) <= CHUNK, "dst bank-degree exceeds chunk size"
        csum = np.concatenate([[0], np.cumsum(dst_cnt)])
        chunks = []  # (dst_lo_i, dst_hi_i) index range into dst_u
        i = 0
        nd = len(dst_u)
        while i < nd:
            # max j with csum[j]-csum[i] <= 128 and j-i <= 16
            j = np.searchsorted(csum, csum[i] + CHUNK, side="right") - 1
            j = min(j, i + MAXD, nd)
            assert j > i
            chunks.append((i, j))
            i = j
        chunks_cb[(c, b)] = (dst_u, csum, chunks, ld, ls, v)

    nbatch = 0
    for (c, b), (_, _, chunks, _, _, _) in chunks_cb.items():
        nbatch = max(nbatch, (len(chunks) + BCH - 1) // BCH)
    nchunks = nbatch * BCH
    zrow = nbatch * SNUM  # staged zero row per bank
    assert zrow <= 32767, f"staged rows {zrow} exceed int16 gather range"

    # build per-core arrays
    plans = []
    for c in range(C):
        gidx = np.zeros((128, NBANK * nbatch * GNUM // 16), np.int16)
        onehot = np.zeros((128, NBANK * nchunks, MAXD), np.float32)
        rbidx = np.zeros((128, NBANK * 2 * (RBH // 16)), np.int16)
        for b in range(NBANK):
            dst_u, csum, chunks, ld, ls, v = chunks_cb[(c, b)]
            # pad = 0: a safe in-bank read whose one-hot column is zero
            # (negative "skip" indices hang the device — twice reproduced)
            gvals = np.zeros(nbatch * GNUM, np.int64)
            rbrow = np.full(NPCP, zrow, np.int64)  # default: zero row
            for k, (i, j) in enumerate(chunks):
                e0, e1 = csum[i], csum[j]
                nsl = e1 - e0
                # order slots by source row for HBM read locality
                perm = np.argsort(ls[e0:e1], kind="stable")
                gvals[k * CHUNK:k * CHUNK + nsl] = ls[e0:e1][perm]
                q, kk = divmod(k, BCH)
                h, ksl = divmod(kk, 4)
                base = q * SNUM + h * 128 + ksl * 32
                rbrow[dst_u[i:j]] = base + np.arange(j - i)
                # one-hot columns: position within chunk's distinct dsts
                colid = np.searchsorted(dst_u[i:j], ld[e0:e1])[perm]
                oh = onehot[:, b * nchunks + k, :]
                oh[np.arange(nsl), colid] = v[e0:e1][perm]
            q0 = b * nbatch
            gidx[:, q0 * (GNUM // 16):(q0 + nbatch) * (GNUM // 16)] = _pack_idx16(
                gvals, nbatch * GNUM // 16)
            for half in range(2):
                col0 = (b * 2 + half) * (RBH // 16)
                rbidx[:, col0:col0 + RBH // 16] = _pack_idx16(
                    rbrow[half * RBH:(half + 1) * RBH], RBH // 16)

        # graph pooling one-hot with 1/cnt
        cnt = np.bincount(bt, minlength=G).astype(np.float64)
        inv = (1.0 / np.maximum(cnt, 1.0)).astype(np.float32)
        gpool = np.zeros((128, NSB, G), np.float32)
        nodes = np.arange(NPC) + c * NPC
        gb = bt[nodes]
        p = np.arange(NPC) % 128
        sb = np.arange(NPC) // 128
        gpool[p, sb, gb] = inv[gb]
        plans.append(dict(gidx=gidx, onehot=onehot, rbidx=rbidx, gpool=gpool))

    return plans, nbatch, nchunks


def _fold_weights(inputs):
    s = 1.0 / np.float32(np.sqrt(1.0 + BN_EPS))
    Ws, bs = inputs["Ws"], inputs["bs"]
    bn_g, bn_b = inputs["bn_g"], inputs["bn_b"]
    conv = []
    for l in range(3):
        sl = (np.asarray(bn_g[l]) * s).astype(np.float32)
        Wp = (np.asarray(Ws[l]) * sl[None, :]).astype(np.float32)
        bp = (np.asarray(bs[l]) * sl + np.asarray(bn_b[l])).astype(np.float32)
        conv.append((Wp, bp))
    s1 = (np.asarray(inputs["fc1_g"]) * s).astype(np.float32)
    W1 = (np.asarray(inputs["fc1_W"]) * s1[None, :]).astype(np.float32)
    b1 = (np.asarray(inputs["fc1_b"]) * s1 + np.asarray(inputs["fc1_bt"])).astype(np.float32)
    s2 = (np.asarray(inputs["fc2_g"]) * s).astype(np.float32)
    W2 = (np.asarray(inputs["fc2_W"]) * s2[None, :]).astype(np.float32)
    b2 = (np.asarray(inputs["fc2_b"]) * s2 + np.asarray(inputs["fc2_bt"])).astype(np.float32)
    Wo = np.asarray(inputs["fco_W"], np.float32)
    bo = np.asarray(inputs["fco_b"], np.float32)
    return conv, (W1, b1), (W2, b2), (Wo, bo)


# ---------------------------------------------------------------- device
def build_bass(nbatch, nchunks):
    import concourse.bacc as bacc
    import concourse.bass as bass
    import concourse.mybir as mybir
    import concourse.tile as tile

    dt = mybir.dt
    nc = bacc.Bacc("TRN2", target_bir_lowering=False, debug=False, num_devices=C,
                   num_swdge_queues=4)

    zrow = nbatch * SNUM

    xT_in = nc.dram_tensor("xT", [F, NPCP], dt.bfloat16, kind="ExternalInput")
    gidx_in = nc.dram_tensor("gidx", [128, NBANK * nbatch * GNUM // 16], dt.int16,
                             kind="ExternalInput")
    oh_in = nc.dram_tensor("onehot", [128, NBANK * nchunks, MAXD], dt.bfloat16,
                           kind="ExternalInput")
    rbidx_in = nc.dram_tensor("rbidx", [128, NBANK * 2 * (RBH // 16)], dt.int16,
                              kind="ExternalInput")
    gpool_in = nc.dram_tensor("gpool", [128, NSB, G], dt.float32, kind="ExternalInput")
    Wc_in = nc.dram_tensor("Wconv", [3, F, F], dt.bfloat16, kind="ExternalInput")
    bc_in = nc.dram_tensor("bconv", [3, 128, F], dt.float32, kind="ExternalInput")
    W1_in = nc.dram_tensor("W1", [F, F], dt.float32, kind="ExternalInput")
    b1_in = nc.dram_tensor("b1", [F, 1], dt.float32, kind="ExternalInput")
    W2_in = nc.dram_tensor("W2", [F, 32], dt.float32, kind="ExternalInput")
    b2_in = nc.dram_tensor("b2", [32, 1], dt.float32, kind="ExternalInput")
    Wo_in = nc.dram_tensor("Wo", [32, 2], dt.float32, kind="ExternalInput")
    bo_in = nc.dram_tensor("bo", [2, 1], dt.float32, kind="ExternalInput")
    ident_in = nc.dram_tensor("ident", [128, 128], dt.float32, kind="ExternalInput")
    out_ext = nc.dram_tensor("out", [2, G], dt.float32, kind="ExternalOutput")

    # table rows are 64 feats duplicated twice (256B bf16) so dma_gather's
    # 256B-min element lands directly in matmul-ready bf16
    shard_d = nc.dram_tensor("shard_d", [NPCP, 2 * F], dt.bfloat16)
    table = nc.dram_tensor("table", [C * NPCP, 2 * F], dt.bfloat16,
                           addr_space="Shared")
    staged = nc.dram_tensor("staged", [NBANK, zrow + 1, F], dt.float32)
    pool_in_d = nc.dram_tensor("pool_in", [F, G], dt.float32)
    pool_out_d = nc.dram_tensor("pool_out", [F, G], dt.float32, addr_space="Shared")

    RG = [list(range(C))]

    with tile.TileContext(nc) as tc:
        with (
            tc.tile_pool(name="persist", bufs=1) as persist,
            tc.tile_pool(name="ht", bufs=2) as htp,
            tc.tile_pool(name="io", bufs=6) as iop,
            tc.tile_pool(name="msgp", bufs=5) as msgp,
            tc.tile_pool(name="stagep", bufs=2) as stagep,
            tc.tile_pool(name="rbp", bufs=1) as rbp,
            tc.tile_pool(name="small", bufs=4) as smallp,
            tc.tile_pool(name="ppsum", bufs=2, space="PSUM") as ppsum,
            tc.tile_pool(name="spsum", bufs=3, space="PSUM") as spsum,
            tc.tile_pool(name="tpsum", bufs=2, space="PSUM") as tpsum,
            tc.tile_pool(name="accpsum", bufs=1, space="PSUM") as accpsum,
        ):
            ident = persist.tile([128, 128], dt.float32)
            nc.sync.dma_start(ident[:], ident_in[:])
            Wc = persist.tile([F, 3 * F], dt.bfloat16)
            nc.sync.dma_start(Wc[:].rearrange("p (l f) -> p l f", l=3),
                              Wc_in[:].rearrange("l p f -> p l f"))
            bc = persist.tile([128, 3 * F], dt.float32)
            nc.sync.dma_start(bc[:].rearrange("p (l f) -> p l f", l=3),
                              bc_in[:].rearrange("l p f -> p l f"))
            gpool = persist.tile([128, NSB * G], dt.float32)
            nc.sync.dma_start(gpool[:].rearrange("p (s g) -> p s g", g=G), gpool_in[:])
            zz = persist.tile([1, F], dt.float32)
            nc.vector.memset(zz[:], 0.0)
            for b in range(NBANK):
                nc.sync.dma_start(staged[b, zrow:zrow + 1, :], zz[:])

            # pre-zero the msg ring slots so slots skipped by padded
            # gathers never hold non-finite garbage (0 * one-hot stays 0)
            for _ in range(5):
                mz = msgp.tile([128, BCH * 2 * F], dt.bfloat16, tag="msg")
                nc.gpsimd.memset(mz[:], 0.0)

            hT = htp.tile([F, NPCP], dt.bfloat16, name="hT0", tag="hT")
            nc.sync.dma_start(hT[:], xT_in[:])

            def proj_half(l, half, hsrc):
                # project one half-shard and allgather it: the AG overlaps
                # the other half's readback/merge and the early banks' batches
                for t in range(half * (NSB // 2), (half + 1) * (NSB // 2)):
                    pp = ppsum.tile([128, F], dt.float32, space="PSUM", tag="pp")
                    nc.tensor.matmul(out=pp[:], lhsT=hsrc[:, t * 128:(t + 1) * 128],
                                     rhs=Wc[:, l * F:(l + 1) * F],
                                     start=True, stop=True)
                    ps = smallp.tile([128, 2 * F], dt.bfloat16, tag="ps")
                    nc.vector.tensor_copy(ps[:, 0:F], pp[:])
                    nc.scalar.activation(ps[:, F:2 * F], pp[:],
                                         mybir.ActivationFunctionType.Copy)
                    nc.sync.dma_start(shard_d[t * 128:(t + 1) * 128, :], ps[:])
                nc.gpsimd.collective_compute(
                    "AllGather", mybir.AluOpType.bypass, replica_groups=RG,
                    ins=[shard_d[half * HALF:(half + 1) * HALF]],
                    outs=[table[half * 8 * HALF:(half + 1) * 8 * HALF]])

            proj_half(0, 0, hT)
            proj_half(0, 1, hT)

            qn = 0
            for l in range(3):
                # ---- gather / one-hot matmul / dense staging
                # (banks 0/1 depend on AG half 0 only, so issue them first;
                # bank interleave spreads the sparse trailing batches)
                for gi in range(2 * nbatch):
                    g = gi % nbatch
                    for b in (0, 1) if gi < nbatch else (2, 3):
                        q = b * nbatch + g
                        git = iop.tile([128, GNUM // 16], dt.int16, tag="git")
                        nc.sync.dma_start(
                            git[:], gidx_in[:, q * (GNUM // 16):(q + 1) * (GNUM // 16)])
                        msg = msgp.tile([128, BCH * 2 * F], dt.bfloat16, tag="msg")
                        nc.gpsimd.dma_gather(
                            out_ap=msg[:].rearrange("p (c f) -> p c f", f=2 * F),
                            in_ap=table[b * BANKR:(b + 1) * BANKR, :],
                            idxs_ap=git[:],
                            num_idxs=GNUM, num_idxs_reg=GNUM, elem_size=2 * F,
                            single_packet=False,
                            queue_num=qn % 4)
                        qn += 1
                        oht = iop.tile([128, BCH * MAXD], dt.bfloat16, tag="oht")
                        nc.sync.dma_start(
                            oht[:].rearrange("p (c m) -> p c m", m=MAXD),
                            oh_in[:, q * BCH:(q + 1) * BCH, :])
                        stg_s = stagep.tile([128, (SNUM // 128) * F], dt.float32,
                                            tag="stg")
                        for h in range(SNUM // 128):
                            sp = spsum.tile([128, F], dt.float32, space="PSUM",
                                            tag="sp")
                            for k in range(4):
                                kk = h * 4 + k
                                nc.tensor.matmul(
                                    out=sp[k * 32:k * 32 + MAXD, :],
                                    lhsT=oht[:, kk * MAXD:(kk + 1) * MAXD],
                                    rhs=msg[:, kk * 2 * F:kk * 2 * F + F],
                                    start=True, stop=True,
                                    tile_position=(0, k * 32))
                            if h % 2 == 0:
                                nc.scalar.activation(
                                    stg_s[:, h * F:(h + 1) * F], sp[:],
                                    mybir.ActivationFunctionType.Copy)
                            else:
                                nc.vector.tensor_copy(
                                    stg_s[:, h * F:(h + 1) * F], sp[:])
                        # on the scalar queue: a staging write waits on this
                        # batch's PSUM copies, and on the sync queue it would
                        # head-of-line block the next batches' index loads
                        nc.scalar.dma_start(
                            staged[b, g * SNUM:(g + 1) * SNUM, :].rearrange(
                                "(h p) f -> p h f", p=128),
                            stg_s[:].rearrange("p (h f) -> p h f", f=F))

                # ---- readback: gather staged rows into dst order, merge
                # banks, bias + relu (+ transpose | pool)
                if l < 2:
                    hT_next = htp.tile([F, NPCP], dt.bfloat16, name=f"hT{l + 1}",
                                       tag="hT")
                else:
                    pacc = accpsum.tile([F, G], dt.float32, space="PSUM")
                for half in range(2):
                    rbt = []
                    for b in range(NBANK):
                        rb = rbp.tile([128, (RBH // 128) * F], dt.float32,
                                      tag=f"rb{b}")
                        col0 = (b * 2 + half) * (RBH // 16)
                        rit = iop.tile([128, RBH // 16], dt.int16, tag="rit")
                        nc.sync.dma_start(
                            rit[:], rbidx_in[:, col0:col0 + RBH // 16])
                        nc.gpsimd.dma_gather(
                            out_ap=rb[:].rearrange("p (c f) -> p c f", f=F),
                            in_ap=staged[b],
                            idxs_ap=rit[:],
                            num_idxs=RBH, num_idxs_reg=RBH, elem_size=F,
                            single_packet=False,
                            queue_num=qn % 4)
                        qn += 1
                        rbt.append(rb)
                    for tl in range(RBH // 128):
                        t = half * (RBH // 128) + tl
                        sl = slice(tl * F, (tl + 1) * F)
                        m0 = smallp.tile([128, F], dt.float32, tag="m0")
                        nc.vector.tensor_add(m0[:], rbt[0][:, sl], rbt[1][:, sl])
                        m1 = smallp.tile([128, F], dt.float32, tag="m1")
                        nc.vector.tensor_add(m1[:], rbt[2][:, sl], rbt[3][:, sl])
                        nc.vector.tensor_add(m0[:], m0[:], m1[:])
                        nc.vector.tensor_add(
                            m0[:], m0[:], bc[:, l * F:(l + 1) * F])
                        nc.vector.tensor_scalar_max(m0[:], m0[:], 0.0)
                        if l < 2:
                            pt = tpsum.tile([F, 128], dt.float32, space="PSUM",
                                            tag="pt")
                            nc.tensor.transpose(pt[:], m0[:], ident[:])
                            nc.scalar.activation(
                                hT_next[:, t * 128:(t + 1) * 128], pt[:],
                                mybir.ActivationFunctionType.Copy)
                        else:
                            nc.tensor.matmul(
                                out=pacc[:], lhsT=m0[:],
                                rhs=gpool[:, t * G:(t + 1) * G],
                                start=(t == 0), stop=(t == NSB - 1))
                    # next layer's projection+AG of this half starts now,
                    # overlapping the other half's readback
                    if l < 2:
                        proj_half(l + 1, half, hT_next)
                if l < 2:
                    hT = hT_next

            # ---- pooled AllReduce + MLP head
            pool_s = smallp.tile([F, G], dt.float32, tag="pool_s")
            nc.vector.tensor_copy(pool_s[:], pacc[:])
            nc.sync.dma_start(pool_in_d[:], pool_s[:])
            nc.gpsimd.collective_compute(
                "AllReduce", mybir.AluOpType.add, replica_groups=RG,
                ins=[pool_in_d[:]], outs=[pool_out_d[:]])
            pooled = smallp.tile([F, G], dt.float32, tag="pooled")
            nc.sync.dma_start(pooled[:], pool_out_d[:])

            W1t = smallp.tile([F, F], dt.float32, tag="W1t")
            nc.sync.dma_start(W1t[:], W1_in[:])
            b1t = smallp.tile([F, 1], dt.float32, tag="b1t")
            nc.sync.dma_start(b1t[:], b1_in[:])
            W2t = smallp.tile([F, 32], dt.float32, tag="W2t")
            nc.sync.dma_start(W2t[:], W2_in[:])
            b2t = smallp.tile([32, 1], dt.float32, tag="b2t")
            nc.sync.dma_start(b2t[:], b2_in[:])
            Wot = smallp.tile([32, 2], dt.float32, tag="Wot")
            nc.sync.dma_start(Wot[:], Wo_in[:])
            bot = smallp.tile([2, 1], dt.float32, tag="bot")
            nc.sync.dma_start(bot[:], bo_in[:])

            h1p = ppsum.tile([F, G], dt.float32, space="PSUM", tag="pp")
            nc.tensor.matmul(out=h1p[:], lhsT=W1t[:], rhs=pooled[:],
                             start=True, stop=True)
            h1 = smallp.tile([F, G], dt.float32, tag="h1")
            nc.scalar.activation(h1[:], h1p[:], mybir.ActivationFunctionType.Relu,
                                 bias=b1t[:])
            h2p = ppsum.tile([32, G], dt.float32, space="PSUM", tag="pp")
            nc.tensor.matmul(out=h2p[:], lhsT=W2t[:], rhs=h1[:],
                             start=True, stop=True)
            h2 = smallp.tile([32, G], dt.float32, tag="h2")
            nc.scalar.activation(h2[:], h2p[:], mybir.ActivationFunctionType.Relu,
                                 bias=b2t[:])
            hop = ppsum.tile([2, G], dt.float32, space="PSUM", tag="pp")
            nc.tensor.matmul(out=hop[:], lhsT=Wot[:], rhs=h2[:],
                             start=True, stop=True)
            outt = smallp.tile([2, G], dt.float32, tag="outt")
            nc.vector.tensor_add(outt[:], hop[:], bot[:].to_broadcast([2, G]))
            nc.sync.dma_start(out_ext[:], outt[:])

    nc.compile()
    return nc


_CACHE = {}


def kernel(**inputs) -> np.ndarray:
    _install_axon_prof_hook()
    import ml_dtypes
    from concourse.bass_utils import run_bass_kernel_spmd

    x = np.asarray(inputs["x"], np.float32)
    plans, nbatch, nchunks = build_plan(
        inputs["edge_index"], inputs["edge_weight"], inputs["batch"])
    conv, (W1, b1), (W2, b2), (Wo, bo) = _fold_weights(inputs)

    key = (nbatch, nchunks)
    if key not in _CACHE:
        _CACHE[key] = build_bass(nbatch, nchunks)
    nc = _CACHE[key]

    Wconv = np.stack([c[0] for c in conv]).astype(ml_dtypes.bfloat16)  # [3, F, F]
    bconv = np.stack([np.broadcast_to(c[1], (128, F)) for c in conv]).copy()
    ident = np.eye(128, dtype=np.float32)

    in_maps = []
    for c in range(C):
        xT = np.zeros((F, NPCP), np.float32)
        xT[:, :NPC] = x[c * NPC:(c + 1) * NPC].T
        in_maps.append({
            "xT": xT.astype(ml_dtypes.bfloat16),
            "gidx": plans[c]["gidx"],
            "onehot": plans[c]["onehot"].astype(ml_dtypes.bfloat16),
            "rbidx": plans[c]["rbidx"],
            "gpool": plans[c]["gpool"],
            "Wconv": Wconv, "bconv": bconv,
            "W1": W1, "b1": b1[:, None],
            "W2": W2, "b2": b2[:, None],
            "Wo": Wo, "bo": bo[:, None],
            "ident": ident,
        })

    trace = bool(int(__import__("os").environ.get("BGNN_TRACE", "0")))
    res = run_bass_kernel_spmd(nc, in_maps, list(range(C)), trace=trace)
    kernel.last_exec_time_ns = res.exec_time_ns
    return np.ascontiguousarray(res.results[0]["out"].T)


kernel.last_exec_time_ns = None
